# revision 3
# baseline (speedup 1.0000x reference)
"""Trainium2 Bass kernel for nn_ContactPredictionHead.

Math: reference computes
    logits[b,i,j,o] = sym_{ij}( (h_i*h_j).Wp[o] + (hd_i - hd_j) + bias[o] )
The difference term is antisymmetric in (i,j), so the symmetrization
cancels it exactly. The output reduces to a weighted gram matrix:
    out[b,i,j,o] = sum_d h[b,i,d] * h[b,j,d] * Wp[o,d] + bias[o]
with Wp = W[:, :D].

Sharding: B=4 batches x O=2 output channels = 8 independent [L,L] gram
matrices -> one per NeuronCore. Each core computes
    C = (hT * w).T @ hT   (contraction over D=1280)
with hT = h[b].T pre-transposed by the host; the w-scaled copy `a` is
produced on device by per-partition tensor_scalar ops (w[d] is constant
along j, and d lives on partitions).

C is symmetric: only upper-triangle 128-blocks are computed on the PE
(136/256 of the full grid); strictly-lower blocks are produced by XBAR
DMA-transposes (SBUF->SBUF, 2-byte dtype) of the staged upper blocks --
zero PE/ACT-engine cost, they ride the underutilized DMA engines.

All data is bfloat16 (input, staged outputs, DRAM output; PSUM
accumulation and bias stay fp32). Measured rel err vs fp64 ~2e-3, well
inside the 2e-2 gate; bf16 halves DMA bytes and quadruples DVE scale
throughput vs the earlier f32r version (106.4 us model).

Schedule: chunk-major waves over 512-col chunks of the L x L grid.
Wave c computes the 4 diagonal groups of rows 4c..4c+3 (widths
512/384/256/128, assembled + mirrored into one [512,512] superblock
DMA) plus full 512-wide groups for all row-quads above (each quad:
one [512,512] supertile DMA + one [512,512] mirror DMA built from
DMA-transposes). Input chunk 0 is split in two t-halves so the PE
starts ~3.5 us in; later chunks stream well ahead of their waves. The
last wave runs its diagonal cluster last: the 128-wide tail group has
no mirror work, keeping the drain tail short. Engine roles: PE matmuls
only; DVE scales + PSUM drains + hole fills; ACT issues transposes and
output DMAs; SP issues input loads.
"""

import contextlib

import numpy as np

B, L, D, O = 4, 2048, 1280, 2
P = 128
DT = D // P          # 10 contraction tiles of 128
NT = 512             # chunk width (= fp32 psum bank)
MT = L // P          # 16 row tiles
NCHUNK = L // NT     # 4 column chunks

# Mirror production: "dmat" = XBAR DMA transpose (SBUF->SBUF, no PE
# cost); "pe" = PE transpose-mode + ACT drain (fallback).
MIRROR = "dmat"

# Diagonal-group widths by row-in-cluster (bf16 moving dim has no
# >=256 restriction, so r=3 shrinks to one 128-block).
DIAG_W = (512, 384, 256, 128)

REPS = 1             # benchmark knob: repeat compute in one NEFF
TRACE = False        # test.py sets True to capture an NTFF profile
LAST_RESULT = None   # BassKernelResults of the most recent run

_nc_cache = {}


def _build_nc():
    key = (MIRROR, REPS)
    if key in _nc_cache:
        return _nc_cache[key]

    import concourse.bass as bass
    import concourse.mybir as mybir
    import concourse.tile as tile
    from concourse import bacc
    from concourse.masks import make_identity

    f32 = mybir.dt.float32
    bf16 = mybir.dt.bfloat16

    nc = bacc.Bacc("TRN2", target_bir_lowering=False, debug=False, num_devices=8)
    ht_dram = nc.dram_tensor("ht", [D, L], bf16, kind="ExternalInput")
    w_dram = nc.dram_tensor("wcol", [P, DT], f32, kind="ExternalInput")
    b_dram = nc.dram_tensor("bias", [P, 1], f32, kind="ExternalInput")
    out_dram = nc.dram_tensor("out", [L, L], bf16, kind="ExternalOutput")

    ht3 = ht_dram[:, :].rearrange("(t p) l -> p t l", p=P)  # [128, 10, 2048]

    with contextlib.ExitStack() as stack:
        tc = stack.enter_context(tile.TileContext(nc))
        data = stack.enter_context(tc.tile_pool(name="data", bufs=1))
        psum = stack.enter_context(
            tc.tile_pool(name="psum", bufs=8 if MIRROR == "dmat" else 7,
                         space="PSUM")
        )
        dpool = stack.enter_context(tc.tile_pool(name="dstage", bufs=2))
        tpool = stack.enter_context(tc.tile_pool(name="tstage", bufs=2))
        fpool = stack.enter_context(tc.tile_pool(name="fstage", bufs=3))
        mpool = stack.enter_context(tc.tile_pool(name="mstage", bufs=2))

        h_sb = data.tile([P, DT, L], bf16)   # raw hT resident
        a_sb = data.tile([P, DT, L], bf16)   # w-scaled copy
        w_sb = data.tile([P, DT], f32)
        b_sb = data.tile([P, 1], f32)
        if MIRROR == "pe":
            psumt = stack.enter_context(
                tc.tile_pool(name="psumt", bufs=1, space="PSUM")
            )
            ident = data.tile([P, P], bf16)
            make_identity(nc, ident[:, :])

        nc.sync.dma_start(w_sb[:, :], w_dram[:, :])
        nc.sync.dma_start(b_sb[:, :], b_dram[:, :])

        def emit_load():
            # chunk 0 in two t-halves so matmul k-chains start early;
            # later chunks as single [128, 10, 512] transfers.
            js = bass.ts(0, NT)
            nc.sync.dma_start(h_sb[:, 0:5, js], ht3[:, 0:5, js])
            nc.sync.dma_start(h_sb[:, 5:10, js], ht3[:, 5:10, js])
            for jc in range(1, NCHUNK):
                js = bass.ts(jc, NT)
                nc.sync.dma_start(h_sb[:, :, js], ht3[:, :, js])

        def emit_scale(jc):
            # a[:, t, chunk] = h[:, t, chunk] * w[t*128+p]; per-partition
            # scalar -> TensorScalarPtr, 4x DVE mode on packed bf16.
            js = bass.ts(jc, NT)
            for t in range(DT):
                nc.vector.tensor_scalar(
                    a_sb[:, t, js], h_sb[:, t, js], w_sb[:, t, None],
                    None, mybir.AluOpType.mult,
                )

        def emit_group(m, s, w, st_dst):
            # one output group: psum = a[:, :, m-block].T @ h[:, :, s:s+w]
            ps = psum.tile([P, NT], f32, name="ps")[:, :w]
            for k in range(DT):
                nc.tensor.matmul(
                    ps,
                    a_sb[:, k, bass.ds(m * P, P)],
                    h_sb[:, k, bass.ds(s, w)],
                    start=(k == 0),
                    stop=(k == DT - 1),
                )
            # PSUM->SBUF drain fused with (per-partition) bias add
            nc.vector.tensor_tensor(
                st_dst, ps, b_sb[:, 0, None].to_broadcast((P, w)),
                mybir.AluOpType.add,
            )

        def emit_mirror(src, dst3):
            # dst3[p, i, c] = src[c, i*128 + p] for i in range(dst3 dim 1)
            if MIRROR == "dmat":
                nc.scalar.dma_start_transpose(dst3, src)
                return
            nblk = dst3.shape[1]
            pt = psumt.tile([P, NT], f32, name="pt")[:, : nblk * P]
            for i in range(nblk):
                nc.tensor.transpose(
                    pt[:, bass.ts(i, P)],
                    src[:, bass.ds(i * P, P)],
                    ident[:, :],
                )
            nc.scalar.activation(
                dst3.rearrange("p t c -> p (t c)"), pt,
                mybir.ActivationFunctionType.Copy,
            )

        def emit_diag_cluster(kc):
            # rows 4kc..4kc+3 against diag chunk kc. Assemble the full
            # [512, 512] diagonal superblock in dstage and write it with
            # one DMA: direct upper parts from matmul drains, lower
            # holes filled from the transposed staging (the superblock
            # is symmetric, so T(direct parts) covers the holes).
            dst_t = dpool.tile([P, 4, NT], bf16, name="dstage")
            tst_t = tpool.tile([P, 4, NT], bf16, name="tstage")
            for r in range(4):
                m = 4 * kc + r
                w = DIAG_W[r]
                soff = NT - w
                emit_group(m, kc * NT + soff, w, dst_t[:, r, soff:])
                if r < 3:
                    # T of slot r's rows (r+1..3 blocks) lands at
                    # tstage[:, rr, r-block] = M[r*128+c, rr*128+p]
                    emit_mirror(
                        dst_t[:, r, (r + 1) * P :],
                        tst_t[:, r + 1 : 4, bass.ts(r, P)],
                    )
            for r in range(1, 4):
                nc.vector.tensor_copy(
                    dst_t[:, r, 0 : r * P], tst_t[:, r, 0 : r * P]
                )
            dst = out_dram[
                bass.ds(kc * NT, NT), bass.ds(kc * NT, NT)
            ].rearrange("(t p) c -> p t c", p=P)
            nc.scalar.dma_start(dst, dst_t)

        def emit_full_wave(c):
            # full 512-wide groups for rows 0..4c-1 against chunk c,
            # batched per 4-row quad into supertile + mirror DMAs.
            mst_t = mpool.tile([P, 4, 4 * c, P], bf16, name="mstage")
            for kq in range(c):
                fst_t = fpool.tile([P, 4, NT], bf16, name="fstage")
                for r in range(4):
                    m = 4 * kq + r
                    emit_group(m, c * NT, NT, fst_t[:, r, :])
                    emit_mirror(fst_t[:, r, :], mst_t[:, :, m, :])
                dst = out_dram[
                    bass.ds(kq * NT, NT), bass.ds(c * NT, NT)
                ].rearrange("(t p) c -> p t c", p=P)
                nc.scalar.dma_start(dst, fst_t)
                mdst = out_dram[
                    bass.ds(c * NT, NT), bass.ds(kq * NT, NT)
                ].rearrange("(t p) (m c) -> p t m c", p=P, c=P)
                nc.scalar.dma_start(mdst, mst_t[:, :, bass.ds(kq * 4, 4), :])

        def emit_all():
            emit_load()
            emit_scale(0)
            emit_diag_cluster(0)
            for c in range(1, NCHUNK):
                emit_scale(c)
                if c < NCHUNK - 1:
                    emit_diag_cluster(c)
                    emit_full_wave(c)
                else:
                    # last wave: full groups first, diag cluster last
                    # (its 128-wide tail group has no mirror work, so
                    # the drain+DMA tail is short)
                    emit_full_wave(c)
                    emit_diag_cluster(c)

        if REPS == 1:
            emit_all()
        else:
            with tc.For_i(0, REPS, 1):
                emit_all()

    nc.compile()
    _nc_cache[key] = nc
    return nc


def kernel(hidden_states, W, b):
    global LAST_RESULT
    import ml_dtypes
    from concourse.bass_utils import run_bass_kernel_spmd

    hidden_states = np.asarray(hidden_states, dtype=np.float32)
    W = np.asarray(W, dtype=np.float32)
    b = np.asarray(b, dtype=np.float32)

    Wp = W[:, :D]                                   # [O, D]
    hT = np.ascontiguousarray(hidden_states.transpose(0, 2, 1)).astype(
        ml_dtypes.bfloat16
    )

    in_maps = []
    for c in range(8):
        bb, o = divmod(c, 2)
        wcol = np.ascontiguousarray(Wp[o].reshape(DT, P).T)  # [P, DT]
        bias = np.full((P, 1), b[o], dtype=np.float32)
        in_maps.append({"ht": hT[bb], "wcol": wcol, "bias": bias})

    nc = _build_nc()
    res = run_bass_kernel_spmd(nc, in_maps, core_ids=list(range(8)), trace=TRACE)
    LAST_RESULT = res

    out = np.empty((B, L, L, O), dtype=np.float32)
    for c in range(8):
        bb, o = divmod(c, 2)
        out[bb, :, :, o] = res.results[c]["out"].astype(np.float32)
    return out
